# revision 23
# baseline (speedup 1.0000x reference)
"""Trainium2 Bass kernel for a w8a16 gated MLP (DeepSeek-style dense MLP).

out = (silu(x @ W0) * (x @ W1)) @ W2,  W* = int8-valued weights * per-128-row-block scales.

Strategy: data-parallel over the 8192 tokens across 8 NeuronCores (1024 tokens/core),
full weights replicated per core. No collectives needed.

Per core:
  phase 1: hT[i, t] = silu(x@W0)^T * (x@W1)^T computed i-tile by i-tile
           (lhsT = W0/W1 column block [128h x 128i], rhs = resident xT [128h x t]),
           PSUM fp32 accumulation over the 32 h-subtiles, SiLU on the scalar engine,
           gate*up on the vector engine, spilled to a DRAM scratch in bf16.
  phase 2 ("outt"): outT[h', t] = W2.T @ hT streamed over 8 512-col h-groups,
           stationary = W2 128x128 block (serves both 512-token halves per load),
           8 PSUM banks, fp32 accumulation over the 112 i-subtiles; host
           transposes the [H, T] per-core result back to [T, H].

Host side only reshapes/casts and applies the dequant scales; all matmul FLOPs run
on-device in bf16 (int8 weight values are exact in bf16).

Post-schedule BIR passes (before nc.compile()):
  _strip_redundant_ldweights: the Tile scheduler materializes an InstLdweights
      before EVERY matmul; consecutive reloads of the identical weights AP are
      dropped (Tile's tile tracking guarantees the SBUF region is stable).
  _fuse_ldweights_into_matmuls: surviving [ldweights W][matmul] pairs become a
      single self-loading matmul (one less PE instruction dispatch per block).
  _merge_pe_incs: Tile puts a sem-inc on every PE instruction; runs of
      wait-free instructions donate their incs to the run's last instruction
      (as one sem-add-imm), eliminating per-instruction EVT_SEM writes.
      Cumulative totals at every flush boundary are preserved, so all
      pre-computed wait thresholds stay correct (waiters can only fire later,
      never earlier; flushes happen before any instruction with novel deps,
      which prevents cross-engine cycles).
"""

import os

import numpy as np
import ml_dtypes

H = 4096          # hidden
I = 14336         # intermediate
BS = 128          # quant blocksize
B, S = 4, 2048
T_FULL = B * S    # 8192 tokens
N_CORES = 8
T = T_FULL // N_CORES   # 1024 tokens per core

P = 128
KO = H // P       # 32  k-subtiles for phase 1
IT = I // P       # 112 i-tiles (phase-1 output tiles / phase-2 k-subtiles)
N1 = 512          # phase-1 moving-operand width (PSUM-bank cap: 512 fp32)
TH = T // N1      # 2   token groups for phase-1 psum
NP = H // 512     # 8   phase-2 n-panels of 512
MT = T // P       # 8   phase-2 m-tiles

BF16 = ml_dtypes.bfloat16
ACT_FN = "Silu"  # sim override hook (CoreSim lacks Silu)
PHASES = (1, 2)  # debug hook: build only selected phases
REPS = 1  # debug hook: on-device repetitions (timing amplification)
SKIP_P2_HT_DMA = False  # debug hook: phase-2 timing diagnostic (wrong results)
LDW_EVERY = 1  # debug hook: issue phase-1 ldweights every Nth k (timing diagnostic, wrong results if >1)
P1_MODE = "pairs"  # "pairs": ldw + 2 mms alternating banks; "chains": per-bank 32-mm accumulation chains
P2_MODE = "outt"  # "base": self-loading mms, out [T,H]; "outt": explicit-ldw + bare mms, out [H,T]
STRIP_LDW = True  # strip scheduler-inserted redundant weight reloads (the per-mm ~53ns tax)
MERGE_INCS = True  # coalesce per-instruction PE sem-incs onto the last instruction of each run
FUSE_LDW = True  # fold each surviving ldweights into its first matmul (self-loading)


def _fuse_ldweights_into_matmuls(nc):
    """Replace [ldweights W][matmul ...] with a single self-loading matmul.

    Run after _strip_redundant_ldweights: each surviving InstLdweights is
    followed (in PE program order) by matmuls using those weights. Setting
    ldweights=True on the first matmul makes the hardware matmul perform the
    weight load itself, saving one instruction dispatch per weight block.
    The ldweights' waits/updates/deps move onto the matmul.
    """
    fused = 0
    for fn in nc.m.functions:
        for blk in fn.blocks:
            insts = blk.instructions
            if len(insts) < 8:
                continue
            drop = set()
            pe_idx = [
                j for j, i in enumerate(insts)
                if i.engine == mybir_ET_PE
                and type(i).__name__ in ("InstLdweights", "InstMatmult")
            ]
            for pos, j in enumerate(pe_idx):
                i = insts[j]
                if type(i).__name__ != "InstLdweights":
                    continue
                if i.is_transpose or i.perf_mode is not None:
                    continue
                if pos + 1 >= len(pe_idx):
                    continue
                nxt = insts[pe_idx[pos + 1]]
                if (
                    type(nxt).__name__ != "InstMatmult"
                    or nxt.ldweights
                    or nxt.is_transpose
                    or str(nxt.ins[1]) != str(i.ins[0])
                ):
                    continue
                nxt.ldweights = True
                try:
                    nxt.merge_dependencies_from(i)
                except Exception:
                    pass
                si, ni = i.sync_info, nxt.sync_info
                if si is not None:
                    if ni is None:
                        nxt.sync_info = si
                    else:
                        if si.on_wait:
                            ni.on_wait = list(ni.on_wait) + list(si.on_wait)
                        for u in si.on_update or []:
                            same = [
                                x for x in ni.on_update
                                if x.sync_type == "semaphore" and u.sync_type == "semaphore"
                                and x.id == u.id
                                and x.update_mode in ("sem-inc", "sem-add-imm")
                                and u.update_mode in ("sem-inc", "sem-add-imm")
                            ]
                            if same:
                                same[0].update_mode = "sem-add-imm"
                                same[0].update_value = (
                                    same[0].update_value + u.update_value
                                )
                            else:
                                ni.on_update = list(ni.on_update) + [u]
                drop.add(j)
                fused += 1
            if drop:
                blk.instructions = [x for j, x in enumerate(insts) if j not in drop]
    return fused


def _merge_pe_incs(nc):
    """Coalesce consecutive PE sem-incs into one update on the run's last inst.

    Tile attaches a sem-inc to every PE instruction for progress tracking;
    each inc is an extra EVT_SEM register write (~26ns serialized). Waits
    elsewhere use cumulative thresholds, so moving incs LATER (onto the last
    instruction of a run of wait-free PE instructions) keeps every waiter
    correct — it can only make a waiter fire later, never earlier. Runs are
    flushed before any PE instruction that itself carries waits or sync
    dependencies, which bounds the added latency and prevents cycles.
    """
    merged = 0
    for fn in nc.m.functions:
        for blk in fn.blocks:
            insts = blk.instructions
            if len(insts) < 8:
                continue
            run = []  # (inst, sem_id, value)
            known = set()  # dep names whose completion a prior wait already proved

            def flush():
                nonlocal merged, run
                if len(run) > 1:
                    by_sem = {}
                    for inst, s, v in run:
                        by_sem.setdefault(s, []).append((inst, v))
                    for s, lst in by_sem.items():
                        if len(lst) < 2:
                            continue
                        total = sum(v for _, v in lst)
                        for inst, _ in lst[:-1]:
                            inst.sync_info.on_update = []
                            merged += 1
                        last_u = lst[-1][0].sync_info.on_update[0]
                        if total > 1:
                            last_u.update_mode = "sem-add-imm"
                            last_u.update_value = total
                run = []

            for i in insts:
                if i.engine != mybir_ET_PE:
                    continue
                nm = type(i).__name__
                si = i.sync_info
                barrier = bool(si is not None and si.on_wait)
                try:
                    sd = tuple(i.sync_dependency_names() or ())
                except Exception:
                    sd = None
                if sd is None:
                    barrier = True
                else:
                    for d in sd:
                        if d not in known:
                            barrier = True
                            known.add(d)
                if nm not in ("InstLdweights", "InstMatmult"):
                    flush()
                    continue
                if barrier:
                    flush()
                ups = list(si.on_update) if si is not None else []
                if (
                    len(ups) == 1
                    and ups[0].sync_type == "semaphore"
                    and ups[0].update_mode in ("sem-inc", "sem-add-imm")
                    and ups[0].update_reg is None
                ):
                    run.append((i, ups[0].id, ups[0].update_value))
                elif ups:
                    flush()
            flush()
    return merged


def _strip_redundant_ldweights(nc):
    """Remove back-to-back InstLdweights that reload the identical weights AP.

    The Tile scheduler materializes a fresh InstLdweights immediately before
    every InstMatmult, even when an explicit ldweights (or a previous
    companion) already loaded the same SBUF region. Those redundant reloads
    serialize with the matmul stream (~53ns each at N=512). Tile's dependency
    tracking guarantees the SBUF region is stable while any reader of the
    tile is pending, so a repeated load of the same AP is a no-op.
    Sync-carrying loads are kept; dropped loads donate their dependency edges
    to the next kept matmul.
    """
    removed = 0
    for fn in nc.m.functions:
        for blk in fn.blocks:
            insts = blk.instructions
            if len(insts) < 8:
                continue
            out = []
            last_w = None
            pending = []  # dropped companions whose deps must move to the next PE inst
            for i in insts:
                nm = type(i).__name__
                if nm == "InstLdweights":
                    if i.is_transpose or i.perf_mode is not None:
                        last_w = None
                        out.append(i)
                        continue
                    key = str(i.ins[0])
                    si = i.sync_info
                    clean = si is None or (not si.on_wait and not si.on_update)
                    if key == last_w and clean:
                        pending.append(i)
                        removed += 1
                        continue
                    last_w = key
                    out.append(i)
                elif nm == "InstMatmult":
                    if i.is_transpose:
                        last_w = None
                    elif i.ldweights:
                        last_w = str(i.ins[1])
                    for c in pending:
                        try:
                            i.merge_dependencies_from(c)
                        except Exception:
                            pass
                    pending = []
                    out.append(i)
                else:
                    out.append(i)
            for c in pending:  # block ended with dropped loads (shouldn't happen)
                out.append(c)
            blk.instructions = out
    return removed

_PROGRAM = None
_last_in_maps = None


def _bare_matmul(nc, mybir, out, rhs, w, start, stop):
    # matmul that reuses the stationary operand already loaded by an explicit
    # nc.tensor.ldweights(w) — skips the per-matmul LDWEIGHTS (~55-70ns each).
    eng = nc.tensor
    ifmap_ap = eng.lower_ap(rhs.opt({0}), opt=False)
    weights_ap = eng.lower_ap(w.opt({0}), opt=False, for_matmul_weights=True)
    out_ap = eng.lower_ap(out)
    return eng.add_instruction(
        mybir.InstMatmult(
            name=nc.get_next_instruction_name(),
            replication_resolution=0,
            replication_shift_amnt=0,
            replication_num_rows=0,
            start_tensor_calc=start,
            stop_tensor_calc=stop,
            ins=[ifmap_ap, weights_ap],
            ldweights=False,
            outs=[out_ap],
            perf_mode=None,
            is_transpose=None,
            ifmap_quant_offset=None,
            weights_quant_offset=None,
            bass_skip_group_check=True,
            tile_position=(0, 0),
            tile_size=(128, 128),
        )
    )


def _build_program():
    import concourse.mybir as mybir
    from concourse import bacc
    from concourse.tile import TileContext

    bf = mybir.dt.bfloat16
    f32 = mybir.dt.float32

    nc = bacc.Bacc(None, target_bir_lowering=False)

    xt = nc.declare_dram_parameter("xt", [KO, P, T], bf, isOutput=False)
    w0 = nc.declare_dram_parameter("w0t", [IT, P, KO, P], bf, isOutput=False)
    w1 = nc.declare_dram_parameter("w1t", [IT, P, KO, P], bf, isOutput=False)
    if P2_MODE == "outt":
        w2 = nc.declare_dram_parameter("w2r", [IT, P, H], bf, isOutput=False)
        out = nc.declare_dram_parameter("outt", [H, T], f32, isOutput=True)
    else:
        w2 = nc.declare_dram_parameter("w2t", [NP, IT, P, 512], bf, isOutput=False)
        out = nc.declare_dram_parameter("out", [T, H], f32, isOutput=True)

    from contextlib import ExitStack, nullcontext

    with TileContext(nc) as tc:
        with (
            tc.tile_pool(name="dram", bufs=1, space="DRAM") as dpool,
            tc.tile_pool(name="xpool", bufs=1) as xpool,
            ExitStack() as _rep_ctx,
        ):
            if REPS > 1:
                _rep_ctx.enter_context(tc.For_i(0, REPS, 1))
            ht = dpool.tile([IT, P, T], bf)

            # resident transposed activations: [p, ko, t]
            xts = xpool.tile([P, KO, T], bf, tag="xts")
            for k in range(KO):
                nc.sync.dma_start(out=xts[:, k, :], in_=xt[k])

            # ---------------- phase 1: gate/up + silu*mul ----------------
            if 1 not in PHASES:
                pass
            else:
             with (
                tc.tile_pool(name="wpool", bufs=3) as wpool,
                tc.tile_pool(name="hpool", bufs=3) as hpool,
                tc.tile_pool(name="spool", bufs=3) as spool,
                tc.tile_pool(name="psum1", bufs=2, space="PSUM") as psum1,
            ):
                for it in range(IT):
                    w0blk = wpool.tile([P, KO, P], bf, tag="w0blk")
                    w1blk = wpool.tile([P, KO, P], bf, tag="w1blk")
                    # split loads so they spread across DMA queues
                    for g in range(4):
                        ks = slice(g * (KO // 4), (g + 1) * (KO // 4))
                        nc.sync.dma_start(out=w0blk[:, ks, :], in_=w0[it, :, ks, :])
                        nc.sync.dma_start(out=w1blk[:, ks, :], in_=w1[it, :, ks, :])

                    psg = [psum1.tile([P, N1], f32, tag=f"pg{th}", name=f"pg{th}") for th in range(TH)]
                    psu = [psum1.tile([P, N1], f32, tag=f"pu{th}", name=f"pu{th}") for th in range(TH)]
                    if P1_MODE == "chains":
                        for mat in range(2):
                            wblk = (w0blk, w1blk)[mat]
                            pst = (psg, psu)[mat]
                            for th in range(TH):
                                for k in range(KO):
                                    nc.tensor.ldweights(wblk[:, k, :])
                                    _bare_matmul(
                                        nc, mybir,
                                        pst[th][:, :],
                                        xts[:, k, th * N1:(th + 1) * N1],
                                        wblk[:, k, :],
                                        start=k == 0,
                                        stop=k == KO - 1,
                                    )
                    else:
                     for k in range(KO):
                        st = k == 0
                        sp = k == KO - 1
                        if k % LDW_EVERY == 0:
                            nc.tensor.ldweights(w0blk[:, k, :])
                        for th in range(TH):
                            _bare_matmul(
                                nc, mybir,
                                psg[th][:, :],
                                xts[:, k, th * N1:(th + 1) * N1],
                                w0blk[:, k, :],
                                start=st,
                                stop=sp,
                            )
                        if k % LDW_EVERY == 0:
                            nc.tensor.ldweights(w1blk[:, k, :])
                        for th in range(TH):
                            _bare_matmul(
                                nc, mybir,
                                psu[th][:, :],
                                xts[:, k, th * N1:(th + 1) * N1],
                                w1blk[:, k, :],
                                start=st,
                                stop=sp,
                            )

                    ht_sb = hpool.tile([P, T], bf, tag="ht_sb")
                    for th in range(TH):
                        sg = spool.tile([P, N1], bf, tag="sg")
                        nc.scalar.activation(
                            sg, psg[th], getattr(mybir.ActivationFunctionType, ACT_FN)
                        )
                        nc.vector.tensor_mul(
                            out=ht_sb[:, th * N1:(th + 1) * N1],
                            in0=sg,
                            in1=psu[th],
                        )
                    for g in range(2):
                        ts_ = slice(g * (T // 2), (g + 1) * (T // 2))
                        nc.sync.dma_start(out=ht[it, :, ts_], in_=ht_sb[:, ts_])

            # ---------------- phase 2: down projection ----------------
            if 2 not in PHASES:
                pass
            elif P2_MODE == "outt":
                # outT[h, t] = W2.T @ h: stationary = W2 128x128 block (one
                # explicit ldw serves both 512-token halves), moving = ht[k].
                with (
                    tc.tile_pool(name="h2pool", bufs=6) as h2pool,
                    tc.tile_pool(name="w2pool", bufs=6) as w2pool,
                    tc.tile_pool(name="opool", bufs=8) as opool,
                    tc.tile_pool(name="psum2", bufs=1, space="PSUM") as psum2,
                ):
                    NT = 4  # 128-col n-tiles per 512-wide group
                    for ng in range(NP):
                        pso = [
                            psum2.tile([P, N1], f32, tag=f"po{j}_{th}", name=f"po{j}_{th}")
                            for j in range(NT) for th in range(TH)
                        ]
                        for k in range(IT):
                            htr = h2pool.tile([P, T], bf, tag="htr")
                            if not SKIP_P2_HT_DMA:
                                nc.scalar.dma_start(out=htr[:, :], in_=ht[k])
                            w2b = w2pool.tile([P, 512], bf, tag="w2b")
                            nc.sync.dma_start(
                                out=w2b, in_=w2[k, :, ng * 512:(ng + 1) * 512]
                            )
                            st = k == 0
                            sp = k == IT - 1
                            for j in range(NT):
                                wsl = w2b[:, j * P:(j + 1) * P]
                                nc.tensor.ldweights(wsl)
                                for th in range(TH):
                                    _bare_matmul(
                                        nc, mybir,
                                        pso[j * TH + th][:, :],
                                        htr[:, th * N1:(th + 1) * N1],
                                        wsl,
                                        start=st,
                                        stop=sp,
                                    )
                        for j in range(NT):
                            for th in range(TH):
                                osb = opool.tile([P, N1], f32, tag="osb")
                                ps = pso[j * TH + th]
                                if th == 0:
                                    nc.scalar.activation(
                                        osb, ps, mybir.ActivationFunctionType.Copy
                                    )
                                else:
                                    nc.vector.tensor_copy(out=osb, in_=ps)
                                nc.sync.dma_start(
                                    out=out[
                                        ng * 512 + j * P:ng * 512 + (j + 1) * P,
                                        th * N1:(th + 1) * N1,
                                    ],
                                    in_=osb,
                                )
            else:
             with (
                tc.tile_pool(name="h2pool", bufs=6) as h2pool,
                tc.tile_pool(name="w2pool", bufs=6) as w2pool,
                tc.tile_pool(name="opool", bufs=4) as opool,
                tc.tile_pool(name="psum2", bufs=1, space="PSUM") as psum2,
            ):
                for n in range(NP):
                    pos = [psum2.tile([P, 512], f32, tag=f"po{m}", name=f"po{m}") for m in range(MT)]
                    for k in range(IT):
                        htr = h2pool.tile([P, T], bf, tag="htr")
                        if not SKIP_P2_HT_DMA:
                            for g in range(2):
                                ts_ = slice(g * (T // 2), (g + 1) * (T // 2))
                                nc.sync.dma_start(out=htr[:, ts_], in_=ht[k, :, ts_])
                        w2b = w2pool.tile([P, 512], bf, tag="w2b")
                        nc.sync.dma_start(out=w2b, in_=w2[n, k])
                        st = k == 0
                        sp = k == IT - 1
                        for m in range(MT):
                            nc.tensor.matmul(
                                pos[m],
                                lhsT=htr[:, m * P:(m + 1) * P],
                                rhs=w2b,
                                start=st,
                                stop=sp,
                            )
                    for m in range(MT):
                        osb = opool.tile([P, 512], f32, tag="osb")
                        nc.vector.tensor_copy(out=osb, in_=pos[m])
                        nc.sync.dma_start(
                            out=out[m * P:(m + 1) * P, n * 512:(n + 1) * 512],
                            in_=osb,
                        )

    if STRIP_LDW:
        n = _strip_redundant_ldweights(nc)
        print(f"stripped {n} redundant ldweights")
    global mybir_ET_PE
    mybir_ET_PE = mybir.EngineType.PE
    if FUSE_LDW:
        n = _fuse_ldweights_into_matmuls(nc)
        print(f"fused {n} ldweights into matmuls")
    if MERGE_INCS:
        n = _merge_pe_incs(nc)
        print(f"merged {n} PE sem-incs")
    nc.compile()
    return nc


def _dequant_bf16(w_int: np.ndarray, s: np.ndarray) -> np.ndarray:
    # w_int [in, out] int32 (int8-valued), s [in//BS, out] fp32 -> bf16 [in, out]
    return (
        w_int.astype(np.float32) * np.repeat(s.astype(np.float32), BS, axis=0)
    ).astype(BF16)


def make_in_maps(x, w0, w1, w2, s0, s1, s2):
    W0 = _dequant_bf16(np.asarray(w0), np.asarray(s0))  # [H, I]
    W1 = _dequant_bf16(np.asarray(w1), np.asarray(s1))  # [H, I]
    W2 = _dequant_bf16(np.asarray(w2), np.asarray(s2))  # [I, H]

    # tiled layouts so every device DMA is contiguous per partition
    w0t = np.ascontiguousarray(W0.reshape(KO, P, IT, P).transpose(2, 1, 0, 3))
    w1t = np.ascontiguousarray(W1.reshape(KO, P, IT, P).transpose(2, 1, 0, 3))
    if P2_MODE == "outt":
        w2m = ("w2r", np.ascontiguousarray(W2.reshape(IT, P, H)))
    else:
        w2m = ("w2t", np.ascontiguousarray(
            W2.reshape(IT, P, NP, 512).transpose(2, 0, 1, 3)))

    x_flat = np.asarray(x, dtype=np.float32).reshape(T_FULL, H)

    in_maps = []
    for c in range(N_CORES):
        xs = x_flat[c * T:(c + 1) * T]                     # [T, H]
        xt_c = np.ascontiguousarray(xs.T).astype(BF16).reshape(KO, P, T)
        in_maps.append({"xt": xt_c, "w0t": w0t, "w1t": w1t, w2m[0]: w2m[1]})
    return in_maps


def assemble(res_list):
    """Per-core result dicts -> full [B, S, H] float32 output."""
    if P2_MODE == "outt":
        parts = [np.asarray(res_list[c]["outt"]).T for c in range(N_CORES)]
    else:
        parts = [np.asarray(res_list[c]["out"]) for c in range(N_CORES)]
    return np.concatenate(parts, axis=0).reshape(B, S, H).astype(np.float32)


def kernel(x, w0, w1, w2, s0, s1, s2, blocksize):
    global _PROGRAM
    from concourse.bass_utils import run_bass_kernel_spmd

    assert int(blocksize) == BS

    in_maps = make_in_maps(x, w0, w1, w2, s0, s1, s2)

    global _last_in_maps
    _last_in_maps = in_maps
    if _PROGRAM is None:
        _PROGRAM = _build_program()

    trace = os.environ.get("KERNEL_TRACE") == "1"
    if trace:
        try:
            from antenv.axon_hooks import get_axon_ntff_profile_hook  # noqa: F401
        except ImportError:
            trace = False
    r = run_bass_kernel_spmd(_PROGRAM, in_maps, list(range(N_CORES)), trace=trace)
    if trace and r.exec_time_ns is not None:
        print(f"HW exec time: {r.exec_time_ns} ns")
    return assemble(r.results)



# revision 30
# speedup vs baseline: 1.2352x; 1.2352x over previous
"""Trainium2 Bass kernel for a w8a16 gated MLP (DeepSeek-style dense MLP).

out = (silu(x @ W0) * (x @ W1)) @ W2,  W* = int8-valued weights * per-128-row-block scales.

Strategy: data-parallel over the 8192 tokens across 8 NeuronCores (1024 tokens/core),
full weights replicated per core. No collectives needed.

Per core:
  phase 1: hT[i, t] = silu(x@W0)^T * (x@W1)^T computed i-tile by i-tile
           (lhsT = W0/W1 column block [128h x 128i], rhs = resident xT [128h x t]),
           PSUM fp32 accumulation over the 32 h-subtiles, SiLU on the scalar engine,
           gate*up on the vector engine, spilled to a DRAM scratch in bf16.
  phase 2 ("outt"): outT[h', t] = W2.T @ hT streamed over 8 512-col h-groups,
           stationary = W2 128x128 block (serves both 512-token halves per load),
           8 PSUM banks, fp32 accumulation over the 112 i-subtiles; host
           transposes the [H, T] per-core result back to [T, H].

Host side only reshapes/casts and applies the dequant scales; all matmul FLOPs run
on-device in bf16 (int8 weight values are exact in bf16).

Post-schedule BIR passes (before nc.compile()):
  _strip_redundant_ldweights: the Tile scheduler materializes an InstLdweights
      before EVERY matmul; consecutive reloads of the identical weights AP are
      dropped (Tile's tile tracking guarantees the SBUF region is stable).
  _fuse_ldweights_into_matmuls: surviving [ldweights W][matmul] pairs become a
      single self-loading matmul (one less PE instruction dispatch per block).
  _merge_pe_incs: Tile puts a sem-inc on every PE instruction; runs of
      wait-free instructions donate their incs to the run's last instruction
      (as one sem-add-imm), eliminating per-instruction EVT_SEM writes.
      Cumulative totals at every flush boundary are preserved, so all
      pre-computed wait thresholds stay correct (waiters can only fire later,
      never earlier; flushes happen before any instruction with novel deps,
      which prevents cross-engine cycles).
"""

import os

import numpy as np
import ml_dtypes

H = 4096          # hidden
I = 14336         # intermediate
BS = 128          # quant blocksize
B, S = 4, 2048
T_FULL = B * S    # 8192 tokens
N_CORES = 8
T = T_FULL // N_CORES   # 1024 tokens per core

P = 128
KO = H // P       # 32  k-subtiles for phase 1
IT = I // P       # 112 i-tiles (phase-1 output tiles / phase-2 k-subtiles)
N1 = 512          # phase-1 moving-operand width (PSUM-bank cap: 512 fp32)
TH = T // N1      # 2   token groups for phase-1 psum
NP = H // 512     # 8   phase-2 n-panels of 512
MT = T // P       # 8   phase-2 m-tiles

BF16 = ml_dtypes.bfloat16
ACT_FN = "Silu"  # sim override hook (CoreSim lacks Silu)
PHASES = (1, 2)  # debug hook: build only selected phases
REPS = 1  # debug hook: on-device repetitions (timing amplification)
SKIP_P2_HT_DMA = False  # debug hook: phase-2 timing diagnostic (wrong results)
LDW_EVERY = 1  # debug hook: issue phase-1 ldweights every Nth k (timing diagnostic, wrong results if >1)
P1_MODE = "pairs"  # "pairs": ldw + 2 mms alternating banks; "chains": per-bank 32-mm accumulation chains
P2_MODE = "outt"  # "base": self-loading mms, out [T,H]; "outt": explicit-ldw + bare mms, out [H,T]
P2_KB = 4  # phase-2 k-tiles per DMA chunk (1 = per-k DMAs); IT must divide evenly
STRIP_LDW = True  # strip scheduler-inserted redundant weight reloads (the per-mm ~53ns tax)
MERGE_INCS = True  # coalesce per-instruction PE sem-incs onto the last instruction of each run
FUSE_LDW = False  # fold each surviving ldweights into its first matmul (self-loading)


def _fuse_ldweights_into_matmuls(nc):
    """Replace [ldweights W][matmul ...] with a single self-loading matmul.

    Run after _strip_redundant_ldweights: each surviving InstLdweights is
    followed (in PE program order) by matmuls using those weights. Setting
    ldweights=True on the first matmul makes the hardware matmul perform the
    weight load itself, saving one instruction dispatch per weight block.
    The ldweights' waits/updates/deps move onto the matmul.
    """
    fused = 0
    for fn in nc.m.functions:
        for blk in fn.blocks:
            insts = blk.instructions
            if len(insts) < 8:
                continue
            drop = set()
            pe_idx = [
                j for j, i in enumerate(insts)
                if i.engine == mybir_ET_PE
                and type(i).__name__ in ("InstLdweights", "InstMatmult")
            ]
            for pos, j in enumerate(pe_idx):
                i = insts[j]
                if type(i).__name__ != "InstLdweights":
                    continue
                if i.is_transpose or i.perf_mode is not None:
                    continue
                if pos + 1 >= len(pe_idx):
                    continue
                nxt = insts[pe_idx[pos + 1]]
                if (
                    type(nxt).__name__ != "InstMatmult"
                    or nxt.ldweights
                    or nxt.is_transpose
                    or str(nxt.ins[1]) != str(i.ins[0])
                ):
                    continue
                nxt.ldweights = True
                try:
                    nxt.merge_dependencies_from(i)
                except Exception:
                    pass
                si, ni = i.sync_info, nxt.sync_info
                if si is not None:
                    if ni is None:
                        nxt.sync_info = si
                    else:
                        if si.on_wait:
                            ni.on_wait = list(ni.on_wait) + list(si.on_wait)
                        for u in si.on_update or []:
                            same = [
                                x for x in ni.on_update
                                if x.sync_type == "semaphore" and u.sync_type == "semaphore"
                                and x.id == u.id
                                and x.update_mode in ("sem-inc", "sem-add-imm")
                                and u.update_mode in ("sem-inc", "sem-add-imm")
                            ]
                            if same:
                                same[0].update_mode = "sem-add-imm"
                                same[0].update_value = (
                                    same[0].update_value + u.update_value
                                )
                            else:
                                ni.on_update = list(ni.on_update) + [u]
                drop.add(j)
                fused += 1
            if drop:
                blk.instructions = [x for j, x in enumerate(insts) if j not in drop]
    return fused


def _merge_pe_incs(nc):
    """Coalesce consecutive PE sem-incs into one update on the run's last inst.

    Tile attaches a sem-inc to every PE instruction for progress tracking;
    each inc is an extra EVT_SEM register write (~26ns serialized). Waits
    elsewhere use cumulative thresholds, so moving incs LATER (onto the last
    instruction of a run of wait-free PE instructions) keeps every waiter
    correct — it can only make a waiter fire later, never earlier. Runs are
    flushed before any PE instruction that itself carries waits or sync
    dependencies, which bounds the added latency and prevents cycles.
    """
    merged = 0
    for fn in nc.m.functions:
        for blk in fn.blocks:
            insts = blk.instructions
            if len(insts) < 8:
                continue
            run = []  # (inst, sem_id, value)
            known = set()  # dep names whose completion a prior wait already proved

            def flush():
                nonlocal merged, run
                if len(run) > 1:
                    by_sem = {}
                    for inst, s, v in run:
                        by_sem.setdefault(s, []).append((inst, v))
                    for s, lst in by_sem.items():
                        if len(lst) < 2:
                            continue
                        total = sum(v for _, v in lst)
                        for inst, _ in lst[:-1]:
                            inst.sync_info.on_update = []
                            merged += 1
                        last_u = lst[-1][0].sync_info.on_update[0]
                        if total > 1:
                            last_u.update_mode = "sem-add-imm"
                            last_u.update_value = total
                run = []

            for i in insts:
                if i.engine != mybir_ET_PE:
                    continue
                nm = type(i).__name__
                si = i.sync_info
                barrier = bool(si is not None and si.on_wait)
                try:
                    sd = tuple(i.sync_dependency_names() or ())
                except Exception:
                    sd = None
                if sd is None:
                    barrier = True
                else:
                    for d in sd:
                        if d not in known:
                            barrier = True
                            known.add(d)
                if nm not in ("InstLdweights", "InstMatmult"):
                    flush()
                    continue
                if barrier:
                    flush()
                ups = list(si.on_update) if si is not None else []
                if (
                    len(ups) == 1
                    and ups[0].sync_type == "semaphore"
                    and ups[0].update_mode in ("sem-inc", "sem-add-imm")
                    and ups[0].update_reg is None
                ):
                    run.append((i, ups[0].id, ups[0].update_value))
                elif ups:
                    flush()
            flush()
    return merged


def _strip_redundant_ldweights(nc):
    """Remove back-to-back InstLdweights that reload the identical weights AP.

    The Tile scheduler materializes a fresh InstLdweights immediately before
    every InstMatmult, even when an explicit ldweights (or a previous
    companion) already loaded the same SBUF region. Those redundant reloads
    serialize with the matmul stream (~53ns each at N=512). Tile's dependency
    tracking guarantees the SBUF region is stable while any reader of the
    tile is pending, so a repeated load of the same AP is a no-op.
    Sync-carrying loads are kept; dropped loads donate their dependency edges
    to the next kept matmul.
    """
    removed = 0
    for fn in nc.m.functions:
        for blk in fn.blocks:
            insts = blk.instructions
            if len(insts) < 8:
                continue
            out = []
            last_w = None
            pending = []  # dropped companions whose deps must move to the next PE inst
            for i in insts:
                nm = type(i).__name__
                if nm == "InstLdweights":
                    if i.is_transpose or i.perf_mode is not None:
                        last_w = None
                        out.append(i)
                        continue
                    key = str(i.ins[0])
                    si = i.sync_info
                    clean = si is None or (not si.on_wait and not si.on_update)
                    if key == last_w and clean:
                        pending.append(i)
                        removed += 1
                        continue
                    last_w = key
                    out.append(i)
                elif nm == "InstMatmult":
                    if i.is_transpose:
                        last_w = None
                    elif i.ldweights:
                        last_w = str(i.ins[1])
                    for c in pending:
                        try:
                            i.merge_dependencies_from(c)
                        except Exception:
                            pass
                    pending = []
                    out.append(i)
                else:
                    out.append(i)
            for c in pending:  # block ended with dropped loads (shouldn't happen)
                out.append(c)
            blk.instructions = out
    return removed

_PROGRAM = None
_last_in_maps = None


def _bare_matmul(nc, mybir, out, rhs, w, start, stop):
    # matmul that reuses the stationary operand already loaded by an explicit
    # nc.tensor.ldweights(w) — skips the per-matmul LDWEIGHTS (~55-70ns each).
    eng = nc.tensor
    ifmap_ap = eng.lower_ap(rhs.opt({0}), opt=False)
    weights_ap = eng.lower_ap(w.opt({0}), opt=False, for_matmul_weights=True)
    out_ap = eng.lower_ap(out)
    return eng.add_instruction(
        mybir.InstMatmult(
            name=nc.get_next_instruction_name(),
            replication_resolution=0,
            replication_shift_amnt=0,
            replication_num_rows=0,
            start_tensor_calc=start,
            stop_tensor_calc=stop,
            ins=[ifmap_ap, weights_ap],
            ldweights=False,
            outs=[out_ap],
            perf_mode=None,
            is_transpose=None,
            ifmap_quant_offset=None,
            weights_quant_offset=None,
            bass_skip_group_check=True,
            tile_position=(0, 0),
            tile_size=(128, 128),
        )
    )


def _build_program():
    import concourse.mybir as mybir
    from concourse import bacc
    from concourse.tile import TileContext

    bf = mybir.dt.bfloat16
    f32 = mybir.dt.float32

    nc = bacc.Bacc(None, target_bir_lowering=False)

    xt = nc.declare_dram_parameter("xt", [KO, P, T], bf, isOutput=False)
    w0 = nc.declare_dram_parameter("w0t", [IT, P, KO, P], bf, isOutput=False)
    w1 = nc.declare_dram_parameter("w1t", [IT, P, KO, P], bf, isOutput=False)
    if P2_MODE == "outt" and P2_KB > 1:
        w2 = nc.declare_dram_parameter(
            "w2k", [IT // P2_KB, P, P2_KB, H], bf, isOutput=False)
        out = nc.declare_dram_parameter("outt", [H, T], f32, isOutput=True)
    elif P2_MODE == "outt":
        w2 = nc.declare_dram_parameter("w2r", [IT, P, H], bf, isOutput=False)
        out = nc.declare_dram_parameter("outt", [H, T], f32, isOutput=True)
    else:
        w2 = nc.declare_dram_parameter("w2t", [NP, IT, P, 512], bf, isOutput=False)
        out = nc.declare_dram_parameter("out", [T, H], f32, isOutput=True)

    from contextlib import ExitStack, nullcontext

    with TileContext(nc) as tc:
        with (
            tc.tile_pool(name="dram", bufs=1, space="DRAM") as dpool,
            tc.tile_pool(name="xpool", bufs=1) as xpool,
            ExitStack() as _rep_ctx,
        ):
            if REPS > 1:
                _rep_ctx.enter_context(tc.For_i(0, REPS, 1))
            if P2_MODE == "outt" and P2_KB > 1:
                ht = dpool.tile([IT // P2_KB, P, P2_KB, T], bf)
            else:
                ht = dpool.tile([IT, P, T], bf)

            # resident transposed activations: [p, ko, t]
            xts = xpool.tile([P, KO, T], bf, tag="xts")
            for k in range(KO):
                nc.sync.dma_start(out=xts[:, k, :], in_=xt[k])

            # ---------------- phase 1: gate/up + silu*mul ----------------
            if 1 not in PHASES:
                pass
            else:
             with (
                tc.tile_pool(name="wpool", bufs=3) as wpool,
                tc.tile_pool(name="hpool", bufs=3) as hpool,
                tc.tile_pool(name="spool", bufs=3) as spool,
                tc.tile_pool(name="psum1", bufs=2, space="PSUM") as psum1,
            ):
                for it in range(IT):
                    w0blk = wpool.tile([P, KO, P], bf, tag="w0blk")
                    w1blk = wpool.tile([P, KO, P], bf, tag="w1blk")
                    # split loads so they spread across DMA queues
                    for g in range(4):
                        ks = slice(g * (KO // 4), (g + 1) * (KO // 4))
                        nc.sync.dma_start(out=w0blk[:, ks, :], in_=w0[it, :, ks, :])
                        nc.sync.dma_start(out=w1blk[:, ks, :], in_=w1[it, :, ks, :])

                    psg = [psum1.tile([P, N1], f32, tag=f"pg{th}", name=f"pg{th}") for th in range(TH)]
                    psu = [psum1.tile([P, N1], f32, tag=f"pu{th}", name=f"pu{th}") for th in range(TH)]
                    if P1_MODE == "chains":
                        for mat in range(2):
                            wblk = (w0blk, w1blk)[mat]
                            pst = (psg, psu)[mat]
                            for th in range(TH):
                                for k in range(KO):
                                    nc.tensor.ldweights(wblk[:, k, :])
                                    _bare_matmul(
                                        nc, mybir,
                                        pst[th][:, :],
                                        xts[:, k, th * N1:(th + 1) * N1],
                                        wblk[:, k, :],
                                        start=k == 0,
                                        stop=k == KO - 1,
                                    )
                    else:
                     for k in range(KO):
                        st = k == 0
                        sp = k == KO - 1
                        if k % LDW_EVERY == 0:
                            nc.tensor.ldweights(w0blk[:, k, :])
                        for th in range(TH):
                            _bare_matmul(
                                nc, mybir,
                                psg[th][:, :],
                                xts[:, k, th * N1:(th + 1) * N1],
                                w0blk[:, k, :],
                                start=st,
                                stop=sp,
                            )
                        if k % LDW_EVERY == 0:
                            nc.tensor.ldweights(w1blk[:, k, :])
                        for th in range(TH):
                            _bare_matmul(
                                nc, mybir,
                                psu[th][:, :],
                                xts[:, k, th * N1:(th + 1) * N1],
                                w1blk[:, k, :],
                                start=st,
                                stop=sp,
                            )

                    ht_sb = hpool.tile([P, T], bf, tag="ht_sb")
                    for th in range(TH):
                        sg = spool.tile([P, N1], bf, tag="sg")
                        nc.scalar.activation(
                            sg, psg[th], getattr(mybir.ActivationFunctionType, ACT_FN)
                        )
                        nc.vector.tensor_mul(
                            out=ht_sb[:, th * N1:(th + 1) * N1],
                            in0=sg,
                            in1=psu[th],
                        )
                    for g in range(2):
                        ts_ = slice(g * (T // 2), (g + 1) * (T // 2))
                        if P2_MODE == "outt" and P2_KB > 1:
                            nc.sync.dma_start(
                                out=ht[it // P2_KB, :, it % P2_KB, ts_],
                                in_=ht_sb[:, ts_],
                            )
                        else:
                            nc.sync.dma_start(out=ht[it, :, ts_], in_=ht_sb[:, ts_])

            # ---------------- phase 2: down projection ----------------
            if 2 not in PHASES:
                pass
            elif P2_MODE == "outt":
                # outT[h, t] = W2.T @ h: stationary = W2 128x128 block (one
                # explicit ldw serves both 512-token halves), moving = ht[k].
                with (
                    tc.tile_pool(name="h2pool", bufs=6) as h2pool,
                    tc.tile_pool(name="w2pool", bufs=6) as w2pool,
                    tc.tile_pool(name="opool", bufs=8) as opool,
                    tc.tile_pool(name="psum2", bufs=1, space="PSUM") as psum2,
                ):
                    NT = 4  # 128-col n-tiles per 512-wide group
                    KB = P2_KB if P2_KB > 1 else 1
                    NKB = IT // KB
                    for ng in range(NP):
                        pso = [
                            psum2.tile([P, N1], f32, tag=f"po{j}_{th}", name=f"po{j}_{th}")
                            for j in range(NT) for th in range(TH)
                        ]
                        for g in range(NKB):
                            if KB > 1:
                                htc = h2pool.tile([P, KB, T], bf, tag="htc")
                                if not SKIP_P2_HT_DMA:
                                    nc.scalar.dma_start(out=htc, in_=ht[g])
                                w2c = w2pool.tile([P, KB, 512], bf, tag="w2c")
                                nc.sync.dma_start(
                                    out=w2c,
                                    in_=w2[g, :, :, ng * 512:(ng + 1) * 512],
                                )
                            else:
                                htc = h2pool.tile([P, 1, T], bf, tag="htc")
                                if not SKIP_P2_HT_DMA:
                                    nc.scalar.dma_start(out=htc[:, 0, :], in_=ht[g])
                                w2c = w2pool.tile([P, 1, 512], bf, tag="w2c")
                                nc.sync.dma_start(
                                    out=w2c[:, 0, :],
                                    in_=w2[g, :, ng * 512:(ng + 1) * 512],
                                )
                            for kk in range(KB):
                                st = g == 0 and kk == 0
                                sp = g == NKB - 1 and kk == KB - 1
                                for j in range(NT):
                                    wsl = w2c[:, kk, j * P:(j + 1) * P]
                                    nc.tensor.ldweights(wsl)
                                    for th in range(TH):
                                        _bare_matmul(
                                            nc, mybir,
                                            pso[j * TH + th][:, :],
                                            htc[:, kk, th * N1:(th + 1) * N1],
                                            wsl,
                                            start=st,
                                            stop=sp,
                                        )
                        for j in range(NT):
                            for th in range(TH):
                                osb = opool.tile([P, N1], f32, tag="osb")
                                ps = pso[j * TH + th]
                                if th == 0:
                                    nc.scalar.activation(
                                        osb, ps, mybir.ActivationFunctionType.Copy
                                    )
                                else:
                                    nc.vector.tensor_copy(out=osb, in_=ps)
                                nc.sync.dma_start(
                                    out=out[
                                        ng * 512 + j * P:ng * 512 + (j + 1) * P,
                                        th * N1:(th + 1) * N1,
                                    ],
                                    in_=osb,
                                )
            else:
             with (
                tc.tile_pool(name="h2pool", bufs=6) as h2pool,
                tc.tile_pool(name="w2pool", bufs=6) as w2pool,
                tc.tile_pool(name="opool", bufs=4) as opool,
                tc.tile_pool(name="psum2", bufs=1, space="PSUM") as psum2,
            ):
                for n in range(NP):
                    pos = [psum2.tile([P, 512], f32, tag=f"po{m}", name=f"po{m}") for m in range(MT)]
                    for k in range(IT):
                        htr = h2pool.tile([P, T], bf, tag="htr")
                        if not SKIP_P2_HT_DMA:
                            for g in range(2):
                                ts_ = slice(g * (T // 2), (g + 1) * (T // 2))
                                nc.sync.dma_start(out=htr[:, ts_], in_=ht[k, :, ts_])
                        w2b = w2pool.tile([P, 512], bf, tag="w2b")
                        nc.sync.dma_start(out=w2b, in_=w2[n, k])
                        st = k == 0
                        sp = k == IT - 1
                        for m in range(MT):
                            nc.tensor.matmul(
                                pos[m],
                                lhsT=htr[:, m * P:(m + 1) * P],
                                rhs=w2b,
                                start=st,
                                stop=sp,
                            )
                    for m in range(MT):
                        osb = opool.tile([P, 512], f32, tag="osb")
                        nc.vector.tensor_copy(out=osb, in_=pos[m])
                        nc.sync.dma_start(
                            out=out[m * P:(m + 1) * P, n * 512:(n + 1) * 512],
                            in_=osb,
                        )

    if STRIP_LDW:
        n = _strip_redundant_ldweights(nc)
        print(f"stripped {n} redundant ldweights")
    global mybir_ET_PE
    mybir_ET_PE = mybir.EngineType.PE
    if FUSE_LDW:
        n = _fuse_ldweights_into_matmuls(nc)
        print(f"fused {n} ldweights into matmuls")
    if MERGE_INCS:
        n = _merge_pe_incs(nc)
        print(f"merged {n} PE sem-incs")
    nc.compile()
    return nc


def _dequant_bf16(w_int: np.ndarray, s: np.ndarray) -> np.ndarray:
    # w_int [in, out] int32 (int8-valued), s [in//BS, out] fp32 -> bf16 [in, out]
    return (
        w_int.astype(np.float32) * np.repeat(s.astype(np.float32), BS, axis=0)
    ).astype(BF16)


def make_in_maps(x, w0, w1, w2, s0, s1, s2):
    W0 = _dequant_bf16(np.asarray(w0), np.asarray(s0))  # [H, I]
    W1 = _dequant_bf16(np.asarray(w1), np.asarray(s1))  # [H, I]
    W2 = _dequant_bf16(np.asarray(w2), np.asarray(s2))  # [I, H]

    # tiled layouts so every device DMA is contiguous per partition
    w0t = np.ascontiguousarray(W0.reshape(KO, P, IT, P).transpose(2, 1, 0, 3))
    w1t = np.ascontiguousarray(W1.reshape(KO, P, IT, P).transpose(2, 1, 0, 3))
    if P2_MODE == "outt" and P2_KB > 1:
        w2m = ("w2k", np.ascontiguousarray(
            W2.reshape(IT // P2_KB, P2_KB, P, H).transpose(0, 2, 1, 3)))
    elif P2_MODE == "outt":
        w2m = ("w2r", np.ascontiguousarray(W2.reshape(IT, P, H)))
    else:
        w2m = ("w2t", np.ascontiguousarray(
            W2.reshape(IT, P, NP, 512).transpose(2, 0, 1, 3)))

    x_flat = np.asarray(x, dtype=np.float32).reshape(T_FULL, H)

    in_maps = []
    for c in range(N_CORES):
        xs = x_flat[c * T:(c + 1) * T]                     # [T, H]
        xt_c = np.ascontiguousarray(xs.T).astype(BF16).reshape(KO, P, T)
        in_maps.append({"xt": xt_c, "w0t": w0t, "w1t": w1t, w2m[0]: w2m[1]})
    return in_maps


def assemble(res_list):
    """Per-core result dicts -> full [B, S, H] float32 output."""
    if P2_MODE == "outt":
        parts = [np.asarray(res_list[c]["outt"]).T for c in range(N_CORES)]
    else:
        parts = [np.asarray(res_list[c]["out"]) for c in range(N_CORES)]
    return np.concatenate(parts, axis=0).reshape(B, S, H).astype(np.float32)


def kernel(x, w0, w1, w2, s0, s1, s2, blocksize):
    global _PROGRAM
    from concourse.bass_utils import run_bass_kernel_spmd

    assert int(blocksize) == BS

    in_maps = make_in_maps(x, w0, w1, w2, s0, s1, s2)

    global _last_in_maps
    _last_in_maps = in_maps
    if _PROGRAM is None:
        _PROGRAM = _build_program()

    trace = os.environ.get("KERNEL_TRACE") == "1"
    if trace:
        try:
            from antenv.axon_hooks import get_axon_ntff_profile_hook  # noqa: F401
        except ImportError:
            trace = False
    r = run_bass_kernel_spmd(_PROGRAM, in_maps, list(range(N_CORES)), trace=trace)
    if trace and r.exec_time_ns is not None:
        print(f"HW exec time: {r.exec_time_ns} ns")
    return assemble(r.results)

